# revision 33
# baseline (speedup 1.0000x reference)
"""Single-head causal attention on 8 TRN2 NeuronCores.

out[b,t,:] = softmax_causal((x Wq^T)(x Wk^T)^T / sqrt(C)) @ (x Wv^T)

Sharding: core = (batch b=core//2, parity p=core%2). Each core owns the
interleaved q-512-blocks g in {p, p+2, p+4, p+6} of its batch.

v5 (v0 197us, v1 156us, v2 112us, v3 107us, ~97us now):
- Startup: concurrent DMAs complete proportionally (SDMA round-robin),
  so only wt_kv + xq0-first-half go out unpaced; every other input is
  gated on the matmul that marks "previous wave consumed" (JIT waves
  on the sync ring). First real matmul ~14us vs 27.7us in v2.
- PE warmup: short bf16 identity-matmul burst during the DMA wait
  accumulates PE-busy time toward the HAM 8/8 clock flip.
- Projections stay in dense bursts (scattering them into the attention
  groups doubled HAM throttle time in experiments); pkv blocks run one
  attention section ahead of first use so the kdup SBUF->SBUF copies
  (gpsimd SWDGE, ~2us latency) land with a full section of slack.
- PV: one K=128 matmul per chunk; acc fits 1 PSUM bank; denominator
  rides the vn ones-column (row 64 of acc); host does the division.
- kdup copies only chunks c0+1..c0+3 (even chunks are always consumed
  from partitions 0:63), one transfer per proj block.
- kv projections: K=128 M=128 matmuls ([Wk;Wv] packed), PSUM pool of
  3 banks shared with the v^T -> v-natural PE transposes.
- q projections: [Wq;Wq] packed so q^T lands duplicated in both
  partition halves.
- Attention: K=64 row-tiled score pairs (k^T dup at partitions
  64-127 for odd chunks), exp batched 2 chunks per ACT op; block
  finalize deferred past the next section's matmuls.
- fp8-DoubleRow projections were tried (86us) but fail the 2e-2
  correctness gate (rel err 3.8e-2): score quantization noise.
"""

import math
import os
import sys

for _p in ("/opt/trn_rl_repo",):
    if _p not in sys.path:
        sys.path.insert(0, _p)

import numpy as np
import ml_dtypes

BF16 = ml_dtypes.bfloat16

B, T, C, H = 4, 4096, 1024, 64
NCORES = 8
SCALE = C ** -0.5

QB = 512
NQB = 4
NKVB = 7
MAINC = 28
NCH = MAINC + 4 * NQB    # 44 kv chunks

_CACHE = {}


def _build_program():
    import concourse.bass as bass
    import concourse.mybir as mybir
    import concourse.tile as tile
    from concourse import bacc
    from concourse.masks import make_identity
    from concourse.tile import add_dep_helper

    f32 = mybir.dt.float32
    bf16 = mybir.dt.bfloat16

    nc = bacc.Bacc("TRN2", target_bir_lowering=False, debug=False)
    xq_d = nc.dram_tensor("xq", [128, NQB * 8 * QB], bf16,
                          kind="ExternalInput")
    xkv_d = nc.dram_tensor("xkv", [128, NKVB * 8 * QB], bf16,
                           kind="ExternalInput")
    wt_d = nc.dram_tensor("wt", [128, 8 * 256], bf16, kind="ExternalInput")
    ind_d = nc.dram_tensor("ind", [128, 1], f32, kind="ExternalInput")
    out_d = nc.dram_tensor("out", [NQB * 65, QB], f32, kind="ExternalOutput")

    with tile.TileContext(nc) as tc:
        with tc.tile_pool(name="persist", bufs=1) as P, \
             tc.tile_pool(name="pw", bufs=4) as W, \
             tc.tile_pool(name="fin", bufs=2) as F, \
             tc.tile_pool(name="pj", bufs=3, space="PSUM") as PJ, \
             tc.tile_pool(name="psc", bufs=2, space="PSUM") as SC, \
             tc.tile_pool(name="pv", bufs=1, space="PSUM") as PV:
            xq_sb = P.tile([128, NQB * 8 * QB], bf16)
            xkv_sb = P.tile([128, NKVB * 8 * QB], bf16)
            wt_sb = P.tile([128, 8 * 256], bf16)
            q2_sb = P.tile([128, NQB * QB], bf16)
            kv_sb = P.tile([128, NCH * 128], bf16)
            khi_sb = P.tile([128, NCH * 128], bf16)
            vn_sb = P.tile([128, NCH * 65], bf16)
            vst_sb = P.tile([128, (NCH // 2) * 128], bf16)
            mask_sb = P.tile([128, 896], bf16)
            idb_sb = P.tile([128, 64], bf16)
            idb2_sb = P.tile([128, 128], bf16)
            idf_sb = P.tile([128, 128], f32)
            ind_sb = P.tile([128, 1], f32)

            # --- constants -------------------------------------------------
            make_identity(nc, idb_sb[64:128, 0:64])
            make_identity(nc, idb2_sb[:, :])
            make_identity(nc, idf_sb[:, :])
            nc.gpsimd.memset(mask_sb[:, :], 1.0)
            nc.gpsimd.affine_select(
                out=mask_sb[:, :], in_=mask_sb[:, :],
                compare_op=mybir.AluOpType.is_ge, fill=0.0,
                base=-384, pattern=[[1, 896]], channel_multiplier=-1)
            nc.gpsimd.memset(vn_sb[:, :], 1.0)

            # --- input DMAs ------------------------------------------------
            # Concurrent DMAs complete proportionally (SDMA engines
            # round-robin across every in-flight transfer), so the only
            # way to get first-needed bytes early is to give them the
            # fabric alone. Unpaced: wt_kv + xq0 first half (+tiny ind)
            # on the scalar ring. Everything else rides the sync ring,
            # each transfer gated on the matmul whose completion means
            # "the previous wave is consumed" (just-in-time waves).
            nc.scalar.dma_start(out=wt_sb[:, 0:1024], in_=wt_d[:, 0:1024])
            HB = 4 * QB      # half an x block (c-chunks 0-3 / 4-7)
            nc.scalar.dma_start(out=xq_sb[:, 0:HB], in_=xq_d[:, 0:HB])
            nc.scalar.dma_start(out=ind_sb[:, :], in_=ind_d[:, :])

            def dma_x(dst, src, b):
                # host pre-swizzled: contiguous per partition -> ~128 large
                # descriptors per transfer instead of ~1024 small ones
                return nc.sync.dma_start(
                    out=dst[:, b * 8 * QB:(b + 1) * 8 * QB],
                    in_=src[:, b * 8 * QB:(b + 1) * 8 * QB])

            pace = {}

            def paced(key, dma_inst):
                pace.setdefault(key, []).append(dma_inst)

            def hit(key, mm_inst):
                for d in pace.pop(key, []):
                    add_dep_helper(mm_inst.ins, d.ins, sync=True,
                                   reason="input dma pacing")

            paced(("d0", 0), nc.sync.dma_start(
                out=xq_sb[:, HB:2 * HB], in_=xq_d[:, HB:2 * HB]))
            paced(("d0", 4), nc.sync.dma_start(
                out=wt_sb[:, 1024:2048], in_=wt_d[:, 1024:2048]))
            paced(("qq0", 0), nc.sync.dma_start(
                out=xkv_sb[:, 0:HB], in_=xkv_d[:, 0:HB]))
            paced(("qq0", 0), nc.sync.dma_start(
                out=xkv_sb[:, HB:2 * HB], in_=xkv_d[:, HB:2 * HB]))
            paced(("p0", 0), dma_x(xkv_sb, xkv_d, 1))
            paced(("p1", 0), dma_x(xkv_sb, xkv_d, 2))
            paced(("p2", 0), dma_x(xq_sb, xq_d, 1))
            paced(("A", 0, 0), dma_x(xkv_sb, xkv_d, 3))
            paced(("A", 0, 2), dma_x(xkv_sb, xkv_d, 4))
            paced(("A", 1, 0), dma_x(xq_sb, xq_d, 2))
            paced(("A", 1, 4), dma_x(xkv_sb, xkv_d, 5))
            paced(("A", 1, 6), dma_x(xkv_sb, xkv_d, 6))
            paced(("A", 2, 0), dma_x(xq_sb, xq_d, 3))

            # --- PE warmup -------------------------------------------------
            # Short bf16 matmul burst (idb is ready early) while the first
            # input DMA is in flight: accumulates PE-busy time toward the
            # HAM 8/8 flip. Writes the PV-pool bank, which nothing needs
            # until attn0's first PV matmul.
            wps = PV.tile([128, 512], f32, tag="acc")
            for _ in range(16):
                nc.tensor.matmul(wps[0:64, 0:64], idb_sb[64:128, 0:64],
                                 idb_sb[64:128, 0:64],
                                 start=True, stop=True)
            # slower f32 matmuls (~430ns each) carry PE activity through
            # the HAM window until the first input lands (~14.8us)
            for _ in range(12):
                nc.tensor.matmul(wps[:, 0:128], idf_sb, idf_sb,
                                 start=True, stop=True)

            # ---------------------------------------------------------------
            def wt_kv(c):
                return wt_sb[:, c * 128:c * 128 + 128]

            def wt_qq(c):
                return wt_sb[:, 1024 + c * 128:1024 + c * 128 + 128]

            def vtr(pi):
                # one [128,128] transpose covers chunk pair (2pi, 2pi+1)
                tp = PJ.tile([128, 128], bf16, tag="pj")
                nc.tensor.transpose(
                    tp, vst_sb[:, pi * 128:(pi + 1) * 128], idb2_sb)
                co, ce = 2 * pi + 1, 2 * pi
                nc.vector.tensor_copy(vn_sb[:, co * 65:co * 65 + 64],
                                      tp[:, 0:64])
                nc.vector.tensor_copy(vn_sb[:, ce * 65:ce * 65 + 64],
                                      tp[:, 64:128])

            def vstage(c0):
                # stage the block's two chunk pairs for vtr: odd v^T
                # crosses partitions via SWDGE (same slack as kdup);
                # even v^T is a partition-aligned DVE copy.
                for pi in (c0 // 2, c0 // 2 + 1):
                    co, ce = 2 * pi + 1, 2 * pi
                    nc.gpsimd.dma_start(
                        out=vst_sb[0:64, pi * 128:(pi + 1) * 128],
                        in_=kv_sb[64:128, co * 128:(co + 1) * 128])
                    nc.vector.tensor_copy(
                        vst_sb[64:128, pi * 128:(pi + 1) * 128],
                        kv_sb[64:128, ce * 128:(ce + 1) * 128])

            def kdup(c0, nch):
                # scores read even chunks from kv_sb[0:64] and odd chunks
                # from khi_sb[64:128]; one transfer spanning chunks
                # c0+1..c0+3 covers both odd chunks.
                nc.gpsimd.dma_start(
                    out=khi_sb[64:128, (c0 + 1) * 128:(c0 + nch) * 128],
                    in_=kv_sb[0:64, (c0 + 1) * 128:(c0 + nch) * 128])

            def proj_steps(rhs_sb, b, out_ap, wfun, hookname, kd_c0):
                """One kv/q projection block as 8 single-matmul closures;
                the last one also evacuates PSUM and issues the kdup."""
                box = [None]

                def mk(c):
                    def go():
                        if c == 0:
                            box[0] = PJ.tile([128, 512], f32, tag="pj",
                                             name="ps_%s_%d" % (hookname or
                                                                "x", b))
                        rhs = rhs_sb[:,
                                     (b * 8 + c) * QB:(b * 8 + c + 1) * QB]
                        mm = nc.tensor.matmul(box[0], wfun(c), rhs,
                                              start=(c == 0), stop=(c == 7))
                        if hookname is not None:
                            hit((hookname, c), mm)
                        if c == 7:
                            nc.vector.tensor_copy(out_ap, box[0])
                            if kd_c0 is not None:
                                kdup(kd_c0, 4)
                                vstage(kd_c0)
                    return go
                return [mk(c) for c in range(8)]

            def steps_pkv(b):
                return proj_steps(
                    xkv_sb, b, kv_sb[:, b * 512:(b + 1) * 512], wt_kv,
                    "p%d" % b if b <= 2 else None, 4 * b)

            def steps_diag(b):
                c0 = MAINC + 4 * b
                return proj_steps(
                    xq_sb, b, kv_sb[:, c0 * 128:(c0 + 4) * 128], wt_kv,
                    "d0" if b == 0 else None, c0)

            def steps_qq(b):
                """q proj with [Wq;Wq]: q^T lands duplicated in both
                partition halves, no cross-partition dup needed."""
                return proj_steps(
                    xq_sb, b, q2_sb[:, b * QB:(b + 1) * QB], wt_qq,
                    "qq0" if b == 0 else None, None)

            fin_state = {}

            def attn_body(i, vtr_list, feed=()):
                nmain = 4 + 8 * i
                S = nmain + 4
                NG = S // 2
                acc = PV.tile([128, 512], f32, tag="acc")
                vti = 0
                feed = list(feed)
                fi = 0

                def chunk_of(s):
                    return s if s < nmain else MAINC + 4 * i + (s - nmain)

                def emit_pv(g, pb):
                    for gj in range(2):
                        s = 2 * g + gj
                        ct = chunk_of(s)
                        nc.tensor.matmul(
                            acc[0:65, 0:512],
                            vn_sb[:, ct * 65:(ct + 1) * 65],
                            pb[:, gj * 512:(gj + 1) * 512],
                            start=(s == 0), stop=(s == S - 1))

                prev = None
                for g in range(NG):
                    sc = SC.tile([128, 1024], f32, tag="sc")
                    for gj in range(2):
                        s = 2 * g + gj
                        ct = chunk_of(s)
                        ksl = slice(ct * 128, (ct + 1) * 128)
                        qsl = slice(i * QB, (i + 1) * QB)
                        osl = slice(gj * 512, (gj + 1) * 512)
                        if gj == 0:
                            mm = nc.tensor.matmul(
                                sc[:, osl], kv_sb[0:64, ksl],
                                q2_sb[0:64, qsl], start=True, stop=True)
                            hit(("A", i, g), mm)
                        else:
                            nc.tensor.matmul(
                                sc[:, osl], khi_sb[64:128, ksl],
                                q2_sb[64:128, qsl], start=True, stop=True)
                    # PV of the previous group: its exp ran during this
                    # group's score matmuls, so the PE never waits on ACT
                    if prev is not None:
                        emit_pv(*prev)
                    nv = min(len(vtr_list) - vti,
                             max(1, -(-len(vtr_list) // NG)))
                    for _ in range(nv):
                        vtr(vtr_list[vti]); vti += 1
                    pb = W.tile([128, 1024], bf16, tag="pb")
                    nc.scalar.activation(
                        pb, sc, mybir.ActivationFunctionType.Exp, scale=SCALE)
                    for gj in range(2):
                        s = 2 * g + gj
                        psl = slice(gj * 512, (gj + 1) * 512)
                        if s >= nmain:
                            d = s - nmain
                            nc.vector.tensor_mul(
                                pb[:, psl], pb[:, psl],
                                mask_sb[:, 384 - d * 128:896 - d * 128])
                        elif s >= nmain - 4:
                            nc.vector.tensor_scalar_mul(
                                pb[:, psl], pb[:, psl], ind_sb[:, 0:1])
                    prev = (g, pb)
                    # interleaved projection work for upcoming sections:
                    # runs in this group's PE slack while ACT does the exp
                    nf = min(len(feed) - fi,
                             max(2, -(-(len(feed) - fi) // (NG - g))))
                    for _ in range(nf):
                        feed[fi](); fi += 1
                emit_pv(*prev)
                assert vti == len(vtr_list) and fi == len(feed)
                ob = F.tile([65, 512], f32, tag="ob")
                nc.vector.tensor_copy(ob, acc[0:65, 0:512])
                fin_state[i] = (acc, ob)

            def attn_fin(i):
                acc, ob = fin_state.pop(i)
                # numerator rows 0:64 + denominator row 64; the host does
                # the division + transpose (cheap there, serial tail here)
                nc.sync.dma_start(
                    out=out_d[i * 65:(i + 1) * 65, :], in_=ob)

            # --- static schedule ------------------------------------------
            # Projections stay in dense bursts (back-to-back N=512 chains
            # keep the HAM clock warm; scattering them into the attention
            # groups doubled throttle time). pkv blocks are projected one
            # attention section ahead of their first score use, so the
            # kdup SBUF->SBUF copies (SWDGE, ~2us) land with slack.
            for s in (steps_diag(0) + steps_qq(0) + steps_pkv(0)
                      + steps_pkv(1) + steps_pkv(2)):
                s()
            attn_body(0, [0, 1, 14, 15])
            for s in steps_diag(1) + steps_qq(1):
                s()
            attn_fin(0)
            for s in steps_pkv(3) + steps_pkv(4):
                s()
            attn_body(1, [2, 3, 4, 5, 16, 17])
            for s in steps_diag(2) + steps_qq(2):
                s()
            attn_fin(1)
            for s in steps_pkv(5) + steps_pkv(6):
                s()
            attn_body(2, [6, 7, 8, 9, 18, 19])
            for s in steps_diag(3) + steps_qq(3):
                s()
            attn_fin(2)
            attn_body(3, [10, 11, 12, 13, 20, 21])
            attn_fin(3)
    nc.compile()
    return nc


def _get_program():
    if "nc" not in _CACHE:
        _CACHE["nc"] = _build_program()
    return _CACHE["nc"]


def _swz(blocks):
    """[1024, 512] col-blocks -> [128, nb*8*512]: partition-contiguous."""
    a = np.stack(blocks, axis=0)                 # [nb, 1024, 512]
    nb = a.shape[0]
    a = a.reshape(nb, 8, 128, QB).transpose(2, 0, 1, 3)
    return np.ascontiguousarray(a.reshape(128, nb * 8 * QB))


def _host_prep(x, Wk, Wq, Wv):
    kv_blocks, qq_blocks = [], []
    for c in range(8):
        sl = slice(128 * c, 128 * c + 128)
        kv_blocks.append(np.concatenate([Wk.T[sl], Wv.T[sl]], axis=1))
        qq_blocks.append(np.concatenate([Wq.T[sl], Wq.T[sl]], axis=1))
    wt = np.concatenate(kv_blocks + qq_blocks, axis=1).astype(BF16)

    xT = [np.ascontiguousarray(x[b].T).astype(BF16) for b in range(B)]
    in_maps = []
    for core in range(NCORES):
        b, p = core // 2, core % 2
        gs = [2 * i + p for i in range(NQB)]
        xq = _swz([xT[b][:, QB * g:QB * (g + 1)] for g in gs])
        xkv = _swz([xT[b][:, QB * g:QB * (g + 1)] for g in range(NKVB)])
        ind = np.full((128, 1), float(p), dtype=np.float32)
        in_maps.append({
            "xq": xq,
            "xkv": xkv,
            "wt": np.ascontiguousarray(wt),
            "ind": ind,
        })
    return in_maps


def _gather(results):
    out = np.zeros((B, T, H), dtype=np.float32)
    for core in range(NCORES):
        b, p = core // 2, core % 2
        shard = np.asarray(results[core]["out"], dtype=np.float32)
        for i in range(NQB):
            g = 2 * i + p
            ob = shard[65 * i:65 * (i + 1), :]          # [65, 512]
            out[b, QB * g:QB * (g + 1), :] = (ob[0:64] / ob[64:65]).T
    return out


def run(x, Wk, Wq, Wv, trace=False, tmpdir=None):
    from concourse.bass_utils import run_bass_kernel_spmd

    nc = _get_program()
    in_maps = _host_prep(x, Wk, Wq, Wv)
    res = run_bass_kernel_spmd(
        nc, in_maps, list(range(NCORES)), trace=trace, tmpdir=tmpdir)
    return _gather(res.results), res


def kernel(x, Wk, Wq, Wv):
    out, _ = run(np.asarray(x, dtype=np.float32),
                 np.asarray(Wk, dtype=np.float32),
                 np.asarray(Wq, dtype=np.float32),
                 np.asarray(Wv, dtype=np.float32))
    return out


# revision 34
# speedup vs baseline: 1.0178x; 1.0178x over previous
"""Single-head causal attention on 8 TRN2 NeuronCores.

out[b,t,:] = softmax_causal((x Wq^T)(x Wk^T)^T / sqrt(C)) @ (x Wv^T)

Sharding: core = (batch b=core//2, parity p=core%2). Each core owns the
interleaved q-512-blocks g in {p, p+2, p+4, p+6} of its batch.

v5 (v0 197us, v1 156us, v2 112us, v3 107us, ~97us now):
- Startup: concurrent DMAs complete proportionally (SDMA round-robin),
  so only wt_kv + xq0-first-half go out unpaced; every other input is
  gated on the matmul that marks "previous wave consumed" (JIT waves
  on the sync ring). First real matmul ~14us vs 27.7us in v2.
- PE warmup: short bf16 identity-matmul burst during the DMA wait
  accumulates PE-busy time toward the HAM 8/8 clock flip.
- Projections stay in dense bursts (scattering them into the attention
  groups doubled HAM throttle time in experiments); pkv blocks run one
  attention section ahead of first use so the kdup SBUF->SBUF copies
  (gpsimd SWDGE, ~2us latency) land with a full section of slack.
- PV: one K=128 matmul per chunk; acc fits 1 PSUM bank; denominator
  rides the vn ones-column (row 64 of acc); host does the division.
- kdup copies only chunks c0+1..c0+3 (even chunks are always consumed
  from partitions 0:63), one transfer per proj block.
- kv projections: K=128 M=128 matmuls ([Wk;Wv] packed), PSUM pool of
  3 banks shared with the v^T -> v-natural PE transposes.
- q projections: [Wq;Wq] packed so q^T lands duplicated in both
  partition halves.
- Attention: K=64 row-tiled score pairs (k^T dup at partitions
  64-127 for odd chunks), exp batched 2 chunks per ACT op; block
  finalize deferred past the next section's matmuls.
- fp8-DoubleRow projections were tried (86us) but fail the 2e-2
  correctness gate (rel err 3.8e-2): score quantization noise.
"""

import math
import os
import sys

for _p in ("/opt/trn_rl_repo",):
    if _p not in sys.path:
        sys.path.insert(0, _p)

import numpy as np
import ml_dtypes

BF16 = ml_dtypes.bfloat16

B, T, C, H = 4, 4096, 1024, 64
NCORES = 8
SCALE = C ** -0.5

QB = 512
NQB = 4
NKVB = 7
MAINC = 28
NCH = MAINC + 4 * NQB    # 44 kv chunks

_CACHE = {}


def _build_program():
    import concourse.bass as bass
    import concourse.mybir as mybir
    import concourse.tile as tile
    from concourse import bacc
    from concourse.masks import make_identity
    from concourse.tile import add_dep_helper

    f32 = mybir.dt.float32
    bf16 = mybir.dt.bfloat16

    nc = bacc.Bacc("TRN2", target_bir_lowering=False, debug=False)
    xq_d = nc.dram_tensor("xq", [128, NQB * 8 * QB], bf16,
                          kind="ExternalInput")
    xkv_d = nc.dram_tensor("xkv", [128, NKVB * 8 * QB], bf16,
                           kind="ExternalInput")
    wt_d = nc.dram_tensor("wt", [128, 8 * 256], bf16, kind="ExternalInput")
    ind_d = nc.dram_tensor("ind", [128, 1], f32, kind="ExternalInput")
    out_d = nc.dram_tensor("out", [NQB * 65, QB], f32, kind="ExternalOutput")

    with tile.TileContext(nc) as tc:
        with tc.tile_pool(name="persist", bufs=1) as P, \
             tc.tile_pool(name="pw", bufs=4) as W, \
             tc.tile_pool(name="fin", bufs=2) as F, \
             tc.tile_pool(name="pj", bufs=3, space="PSUM") as PJ, \
             tc.tile_pool(name="psc", bufs=2, space="PSUM") as SC, \
             tc.tile_pool(name="pv", bufs=1, space="PSUM") as PV:
            xq_sb = P.tile([128, NQB * 8 * QB], bf16)
            xkv_sb = P.tile([128, NKVB * 8 * QB], bf16)
            wt_sb = P.tile([128, 8 * 256], bf16)
            q2_sb = P.tile([128, NQB * QB], bf16)
            kv_sb = P.tile([128, NCH * 128], bf16)
            khi_sb = P.tile([128, NCH * 128], bf16)
            vn_sb = P.tile([128, NCH * 65], bf16)
            mask_sb = P.tile([128, 896], bf16)
            idb_sb = P.tile([128, 64], bf16)
            idf_sb = P.tile([128, 128], f32)
            ind_sb = P.tile([128, 1], f32)

            # --- constants -------------------------------------------------
            make_identity(nc, idb_sb[64:128, 0:64])
            make_identity(nc, idf_sb[:, :])
            nc.gpsimd.memset(mask_sb[:, :], 1.0)
            nc.gpsimd.affine_select(
                out=mask_sb[:, :], in_=mask_sb[:, :],
                compare_op=mybir.AluOpType.is_ge, fill=0.0,
                base=-384, pattern=[[1, 896]], channel_multiplier=-1)
            nc.gpsimd.memset(vn_sb[:, :], 1.0)

            # --- input DMAs ------------------------------------------------
            # Concurrent DMAs complete proportionally (SDMA engines
            # round-robin across every in-flight transfer), so the only
            # way to get first-needed bytes early is to give them the
            # fabric alone. Unpaced: wt_kv + xq0 first half (+tiny ind)
            # on the scalar ring. Everything else rides the sync ring,
            # each transfer gated on the matmul whose completion means
            # "the previous wave is consumed" (just-in-time waves).
            nc.scalar.dma_start(out=wt_sb[:, 0:1024], in_=wt_d[:, 0:1024])
            HB = 4 * QB      # half an x block (c-chunks 0-3 / 4-7)
            nc.scalar.dma_start(out=xq_sb[:, 0:HB], in_=xq_d[:, 0:HB])
            nc.scalar.dma_start(out=ind_sb[:, :], in_=ind_d[:, :])

            def dma_x(dst, src, b):
                # host pre-swizzled: contiguous per partition -> ~128 large
                # descriptors per transfer instead of ~1024 small ones
                return nc.sync.dma_start(
                    out=dst[:, b * 8 * QB:(b + 1) * 8 * QB],
                    in_=src[:, b * 8 * QB:(b + 1) * 8 * QB])

            pace = {}

            def paced(key, dma_inst):
                pace.setdefault(key, []).append(dma_inst)

            def hit(key, mm_inst):
                for d in pace.pop(key, []):
                    add_dep_helper(mm_inst.ins, d.ins, sync=True,
                                   reason="input dma pacing")

            paced(("d0", 0), nc.sync.dma_start(
                out=xq_sb[:, HB:2 * HB], in_=xq_d[:, HB:2 * HB]))
            paced(("d0", 4), nc.sync.dma_start(
                out=wt_sb[:, 1024:2048], in_=wt_d[:, 1024:2048]))
            paced(("qq0", 0), nc.sync.dma_start(
                out=xkv_sb[:, 0:HB], in_=xkv_d[:, 0:HB]))
            paced(("qq0", 0), nc.sync.dma_start(
                out=xkv_sb[:, HB:2 * HB], in_=xkv_d[:, HB:2 * HB]))
            paced(("p0", 0), dma_x(xkv_sb, xkv_d, 1))
            paced(("p1", 0), dma_x(xkv_sb, xkv_d, 2))
            paced(("p2", 0), dma_x(xq_sb, xq_d, 1))
            paced(("A", 0, 0), dma_x(xkv_sb, xkv_d, 3))
            paced(("A", 0, 2), dma_x(xkv_sb, xkv_d, 4))
            paced(("A", 1, 0), dma_x(xq_sb, xq_d, 2))
            paced(("A", 1, 4), dma_x(xkv_sb, xkv_d, 5))
            paced(("A", 1, 6), dma_x(xkv_sb, xkv_d, 6))
            paced(("A", 2, 0), dma_x(xq_sb, xq_d, 3))

            # --- PE warmup -------------------------------------------------
            # Short bf16 matmul burst (idb is ready early) while the first
            # input DMA is in flight: accumulates PE-busy time toward the
            # HAM 8/8 flip. Writes the PV-pool bank, which nothing needs
            # until attn0's first PV matmul.
            wps = PV.tile([128, 512], f32, tag="acc")
            for _ in range(16):
                nc.tensor.matmul(wps[0:64, 0:64], idb_sb[64:128, 0:64],
                                 idb_sb[64:128, 0:64],
                                 start=True, stop=True)
            # slower f32 matmuls (~430ns each) carry PE activity through
            # the HAM window until the first input lands (~14.8us)
            for _ in range(12):
                nc.tensor.matmul(wps[:, 0:128], idf_sb, idf_sb,
                                 start=True, stop=True)

            # ---------------------------------------------------------------
            def wt_kv(c):
                return wt_sb[:, c * 128:c * 128 + 128]

            def wt_qq(c):
                return wt_sb[:, 1024 + c * 128:1024 + c * 128 + 128]

            def vtr(ct):
                tp = PJ.tile([128, 64], bf16, tag="pj")
                nc.tensor.transpose(
                    tp, kv_sb[64:128, ct * 128:(ct + 1) * 128],
                    idb_sb[64:128, 0:64])
                nc.vector.tensor_copy(vn_sb[:, ct * 65:ct * 65 + 64], tp)

            def kdup(c0, nch):
                # scores read even chunks from kv_sb[0:64] and odd chunks
                # from khi_sb[64:128]; one transfer spanning chunks
                # c0+1..c0+3 covers both odd chunks.
                nc.gpsimd.dma_start(
                    out=khi_sb[64:128, (c0 + 1) * 128:(c0 + nch) * 128],
                    in_=kv_sb[0:64, (c0 + 1) * 128:(c0 + nch) * 128])

            def proj_steps(rhs_sb, b, out_ap, wfun, hookname, kd_c0):
                """One kv/q projection block as 8 single-matmul closures;
                the last one also evacuates PSUM and issues the kdup."""
                box = [None]

                def mk(c):
                    def go():
                        if c == 0:
                            box[0] = PJ.tile([128, 512], f32, tag="pj",
                                             name="ps_%s_%d" % (hookname or
                                                                "x", b))
                        rhs = rhs_sb[:,
                                     (b * 8 + c) * QB:(b * 8 + c + 1) * QB]
                        mm = nc.tensor.matmul(box[0], wfun(c), rhs,
                                              start=(c == 0), stop=(c == 7))
                        if hookname is not None:
                            hit((hookname, c), mm)
                        if c == 7:
                            nc.vector.tensor_copy(out_ap, box[0])
                            if kd_c0 is not None:
                                kdup(kd_c0, 4)
                    return go
                return [mk(c) for c in range(8)]

            def steps_pkv(b):
                return proj_steps(
                    xkv_sb, b, kv_sb[:, b * 512:(b + 1) * 512], wt_kv,
                    "p%d" % b if b <= 2 else None, 4 * b)

            def steps_diag(b):
                c0 = MAINC + 4 * b
                return proj_steps(
                    xq_sb, b, kv_sb[:, c0 * 128:(c0 + 4) * 128], wt_kv,
                    "d0" if b == 0 else None, c0)

            def steps_qq(b):
                """q proj with [Wq;Wq]: q^T lands duplicated in both
                partition halves, no cross-partition dup needed."""
                return proj_steps(
                    xq_sb, b, q2_sb[:, b * QB:(b + 1) * QB], wt_qq,
                    "qq0" if b == 0 else None, None)

            fin_state = {}

            def attn_body(i, vtr_list, feed=()):
                nmain = 4 + 8 * i
                S = nmain + 4
                NG = S // 2
                acc = PV.tile([128, 512], f32, tag="acc")
                vti = 0
                feed = list(feed)
                fi = 0

                def chunk_of(s):
                    return s if s < nmain else MAINC + 4 * i + (s - nmain)

                def emit_pv(g, pb):
                    for gj in range(2):
                        s = 2 * g + gj
                        ct = chunk_of(s)
                        nc.tensor.matmul(
                            acc[0:65, 0:512],
                            vn_sb[:, ct * 65:(ct + 1) * 65],
                            pb[:, gj * 512:(gj + 1) * 512],
                            start=(s == 0), stop=(s == S - 1))

                prev = None
                for g in range(NG):
                    sc = SC.tile([128, 1024], f32, tag="sc")
                    for gj in range(2):
                        s = 2 * g + gj
                        ct = chunk_of(s)
                        ksl = slice(ct * 128, (ct + 1) * 128)
                        qsl = slice(i * QB, (i + 1) * QB)
                        osl = slice(gj * 512, (gj + 1) * 512)
                        if gj == 0:
                            mm = nc.tensor.matmul(
                                sc[:, osl], kv_sb[0:64, ksl],
                                q2_sb[0:64, qsl], start=True, stop=True)
                            hit(("A", i, g), mm)
                        else:
                            nc.tensor.matmul(
                                sc[:, osl], khi_sb[64:128, ksl],
                                q2_sb[64:128, qsl], start=True, stop=True)
                    # PV of the previous group: its exp ran during this
                    # group's score matmuls, so the PE never waits on ACT
                    if prev is not None:
                        emit_pv(*prev)
                    nv = min(len(vtr_list) - vti,
                             max(1, -(-len(vtr_list) // NG)))
                    for _ in range(nv):
                        vtr(vtr_list[vti]); vti += 1
                    pb = W.tile([128, 1024], bf16, tag="pb")
                    nc.scalar.activation(
                        pb, sc, mybir.ActivationFunctionType.Exp, scale=SCALE)
                    for gj in range(2):
                        s = 2 * g + gj
                        psl = slice(gj * 512, (gj + 1) * 512)
                        if s >= nmain:
                            d = s - nmain
                            nc.vector.tensor_mul(
                                pb[:, psl], pb[:, psl],
                                mask_sb[:, 384 - d * 128:896 - d * 128])
                        elif s >= nmain - 4:
                            nc.vector.tensor_scalar_mul(
                                pb[:, psl], pb[:, psl], ind_sb[:, 0:1])
                    prev = (g, pb)
                    # interleaved projection work for upcoming sections:
                    # runs in this group's PE slack while ACT does the exp
                    nf = min(len(feed) - fi,
                             max(2, -(-(len(feed) - fi) // (NG - g))))
                    for _ in range(nf):
                        feed[fi](); fi += 1
                emit_pv(*prev)
                assert vti == len(vtr_list) and fi == len(feed)
                ob = F.tile([65, 512], f32, tag="ob")
                nc.vector.tensor_copy(ob, acc[0:65, 0:512])
                fin_state[i] = (acc, ob)

            def attn_fin(i):
                acc, ob = fin_state.pop(i)
                # numerator rows 0:64 + denominator row 64; the host does
                # the division + transpose (cheap there, serial tail here)
                nc.sync.dma_start(
                    out=out_d[i * 65:(i + 1) * 65, :], in_=ob)

            # --- static schedule ------------------------------------------
            # Projections stay in dense bursts (back-to-back N=512 chains
            # keep the HAM clock warm; scattering them into the attention
            # groups doubled throttle time). pkv blocks are projected one
            # attention section ahead of their first score use, so the
            # kdup SBUF->SBUF copies (SWDGE, ~2us) land with slack.
            for s in (steps_diag(0) + steps_qq(0) + steps_pkv(0)
                      + steps_pkv(1) + steps_pkv(2)):
                s()
            attn_body(0, [0, 1, 2, 3, 28, 29, 30, 31])
            for s in steps_diag(1) + steps_qq(1):
                s()
            attn_fin(0)
            for s in steps_pkv(3) + steps_pkv(4):
                s()
            attn_body(1, [4, 5, 6, 7, 8, 9, 10, 11, 32, 33, 34, 35])
            for s in steps_diag(2) + steps_qq(2):
                s()
            attn_fin(1)
            for s in steps_pkv(5) + steps_pkv(6):
                s()
            attn_body(2, [12, 13, 14, 15, 16, 17, 18, 19, 36, 37, 38, 39])
            for s in steps_diag(3) + steps_qq(3):
                s()
            attn_fin(2)
            attn_body(3, [20, 21, 22, 23, 24, 25, 26, 27, 40, 41, 42, 43])
            attn_fin(3)
    nc.compile()
    return nc


def _get_program():
    if "nc" not in _CACHE:
        _CACHE["nc"] = _build_program()
    return _CACHE["nc"]


def _swz(blocks):
    """[1024, 512] col-blocks -> [128, nb*8*512]: partition-contiguous."""
    a = np.stack(blocks, axis=0)                 # [nb, 1024, 512]
    nb = a.shape[0]
    a = a.reshape(nb, 8, 128, QB).transpose(2, 0, 1, 3)
    return np.ascontiguousarray(a.reshape(128, nb * 8 * QB))


def _host_prep(x, Wk, Wq, Wv):
    kv_blocks, qq_blocks = [], []
    for c in range(8):
        sl = slice(128 * c, 128 * c + 128)
        kv_blocks.append(np.concatenate([Wk.T[sl], Wv.T[sl]], axis=1))
        qq_blocks.append(np.concatenate([Wq.T[sl], Wq.T[sl]], axis=1))
    wt = np.concatenate(kv_blocks + qq_blocks, axis=1).astype(BF16)

    xT = [np.ascontiguousarray(x[b].T).astype(BF16) for b in range(B)]
    in_maps = []
    for core in range(NCORES):
        b, p = core // 2, core % 2
        gs = [2 * i + p for i in range(NQB)]
        xq = _swz([xT[b][:, QB * g:QB * (g + 1)] for g in gs])
        xkv = _swz([xT[b][:, QB * g:QB * (g + 1)] for g in range(NKVB)])
        ind = np.full((128, 1), float(p), dtype=np.float32)
        in_maps.append({
            "xq": xq,
            "xkv": xkv,
            "wt": np.ascontiguousarray(wt),
            "ind": ind,
        })
    return in_maps


def _gather(results):
    out = np.zeros((B, T, H), dtype=np.float32)
    for core in range(NCORES):
        b, p = core // 2, core % 2
        shard = np.asarray(results[core]["out"], dtype=np.float32)
        for i in range(NQB):
            g = 2 * i + p
            ob = shard[65 * i:65 * (i + 1), :]          # [65, 512]
            out[b, QB * g:QB * (g + 1), :] = (ob[0:64] / ob[64:65]).T
    return out


def run(x, Wk, Wq, Wv, trace=False, tmpdir=None):
    from concourse.bass_utils import run_bass_kernel_spmd

    nc = _get_program()
    in_maps = _host_prep(x, Wk, Wq, Wv)
    res = run_bass_kernel_spmd(
        nc, in_maps, list(range(NCORES)), trace=trace, tmpdir=tmpdir)
    return _gather(res.results), res


def kernel(x, Wk, Wq, Wv):
    out, _ = run(np.asarray(x, dtype=np.float32),
                 np.asarray(Wk, dtype=np.float32),
                 np.asarray(Wq, dtype=np.float32),
                 np.asarray(Wv, dtype=np.float32))
    return out
